# revision 30
# baseline (speedup 1.0000x reference)
"""Block-sparse attention (nn_BlockSparseAttention) on 8 TRN2 NeuronCores.

Strategy v2: head-parallel attention (2 heads/core), contraction-sharded
o_proj with NO collectives — each core computes the partial o_proj for
its 2 heads over ALL 2048 output dims and the host sums the 8 partials
(the unshard step). Per core, bf16 on the TensorEngine, f32 PSUM:
  1. QKV projections with the weight stationary across 4 q-chunk
     accumulators (1 LDWEIGHTS per 4 matmuls), RoPE fused into the
     PSUM->SBUF eviction (partition-shifted PSUM reads) on DVE.
  2. Attention per (head, q-chunk) unit in [keys, q] orientation,
     software-pipelined one unit deep so the Scalar-engine exp of unit
     u overlaps the PV/den matmuls of unit u-1. Reference mask
     semantics (masked scores = 0 => exp = 1) via decomposition:
     unmasked exp straight from PSUM (ACT), bf16 mask zeroes unselected
     blocks (DVE), and the "+1 per masked key" terms restored by two
     tiny matmuls (V block-sums x complement mask; 64 x complement
     count) accumulated into the same PSUM groups.
  3. o_proj partials per chunk (16 row strips x 2 head-chunks of
     contraction), evicted bf16 and DMA'd out; no cross-core deps.
Host: input prep (top-k block mask, RoPE tables, layouts), final
sum-of-partials + transpose.
"""
import sys

if "/opt/trn_rl_repo" not in sys.path:
    sys.path.insert(0, "/opt/trn_rl_repo")

import numpy as np
import ml_dtypes

import concourse.bass as bass
import concourse.tile as tile
import concourse.mybir as mybir
from concourse import bacc
from concourse.bass_utils import run_bass_kernel_spmd

# problem constants (hardcoded per harness contract)
B, S, HID = 1, 2048, 2048
NH, HD, BS = 16, 128, 64
RATIO = 0.5
THETA = 10000.0
NCORES = 8
HPC = NH // NCORES          # heads per core = 2
P = 128                     # partitions
CH = HID // P               # contraction chunks = 16
KT = S // P                 # key tiles = 16
FB = 512                    # free-dim block (psum bank)
QC = S // FB                # q chunks = 4
NQB = S // BS               # 32 blocks per side
QB_PER_FB = FB // BS        # 8 q-blocks per 512 chunk
KTB = 2                     # key tiles per exp/mask op
NSTRIP = HID // P           # o_proj output strips = 16
CSUB = 4                    # hT / weight DMA sub-chunks

BF = mybir.dt.bfloat16
F32 = mybir.dt.float32

_CACHE = {}


def _build():
    nc = bacc.Bacc("TRN2", target_bir_lowering=False, debug=False,
                   num_devices=NCORES)

    hT = nc.dram_tensor("hT", [QC, P, CH, FB], BF, kind="ExternalInput").ap()
    wq = nc.dram_tensor("wq", [HPC, P, CH, P], BF, kind="ExternalInput").ap()
    wk = nc.dram_tensor("wk", [HPC, P, CH, P], BF, kind="ExternalInput").ap()
    wv = nc.dram_tensor("wv", [HPC, P, CH, P], BF, kind="ExternalInput").ap()
    wo = nc.dram_tensor("wo", [P, HPC, NSTRIP, P], BF, kind="ExternalInput").ap()
    cosT = nc.dram_tensor("cosT", [P, S], F32, kind="ExternalInput").ap()
    sinT = nc.dram_tensor("sinT", [P, S], F32, kind="ExternalInput").ap()  # pre-signed
    binT = nc.dram_tensor("binT", [P, HPC, KT, NQB], BF, kind="ExternalInput").ap()
    binN = nc.dram_tensor("binN", [NQB, HPC, NQB], BF, kind="ExternalInput").ap()
    out = nc.dram_tensor("out", [NSTRIP, P, S], BF, kind="ExternalOutput").ap()

    with tile.TileContext(nc) as tc:
        with tc.tile_pool(name="cp", bufs=1) as cp:
            QTr = cp.tile([P, HPC, S], BF, name="QTr")
            KTr = cp.tile([P, HPC, S], BF, name="KTr")
            V_sbs = [cp.tile([P, KT, P], BF, name=f"V_h{h}")
                     for h in range(HPC)]
            corrT_sb = cp.tile([P, HPC, P], BF, name="corrT_sb")
            at_sb = cp.tile([P, QC, HPC, FB], BF, name="at_sb")
            wo_sb = cp.tile([P, HPC, NSTRIP, P], BF, name="wo_sb")
            bin_sb = cp.tile([P, HPC, KT, NQB], BF, name="bin_sb")
            binN_sb = cp.tile([NQB, HPC, NQB], BF, name="binN_sb")
            ones_sb = cp.tile([P, P], BF, name="ones_sb")
            c64_sb = cp.tile([NQB, P], BF, name="c64_sb")

            # ---------------- QKV + RoPE (phase-scoped pools) ----------------
            qp = tc.alloc_tile_pool(name="qp", bufs=2)
            pq = tc.alloc_tile_pool(name="pq", bufs=1, space="PSUM")

            # weight chunks: [P, CSUB, P] pieces, 3 projs x 2 heads x 4 chunks
            # group order: V groups LAST -- their evictions are cheap ACT
            # copies (DMA transposes ride the idle sync queue), so attention
            # isn't gated on the slow DVE RoPE evictions of the final groups
            w_drams = {"q": wq, "k": wk, "v": wv}
            groups = [(0, "q"), (0, "k"), (1, "q"), (1, "k"), (0, "v"), (1, "v")]
            w_sbs = {}
            CW = CH // CSUB  # 4 c's per weight chunk
            # first group's weights on the scalar queue, ahead of everything
            for gi, (h, pr) in enumerate(groups[:2]):
                chunks = []
                for cs in range(CSUB):
                    w_c = qp.tile([P, CW, P], BF, name=f"w_{pr}{h}_{cs}",
                                  tag="w_sb", bufs=3 * CSUB)
                    nc.scalar.dma_start(
                        w_c[:], w_drams[pr][h, :, cs * CW:(cs + 1) * CW, :])
                    chunks.append(w_c)
                w_sbs[(h, pr)] = chunks
            # hidden^T: queue per qc, qc-major so the first (qc-sequential)
            # group streams as chunks arrive
            hT_engs = [nc.sync, nc.scalar, nc.gpsimd, nc.sync]
            hT_sbs = {}
            for qc in range(QC):
                for cs in range(CSUB):
                    hT_c = qp.tile([P, CW, FB], BF, name=f"hT_{qc}_{cs}",
                                   bufs=1)
                    hT_engs[qc].dma_start(
                        hT_c[:], hT[qc, :, cs * CW:(cs + 1) * CW, :])
                    hT_sbs[(qc, cs)] = hT_c
            # RoPE tables after the first weights + hT qc1 on scalar
            cos_sb = qp.tile([P, S], F32, name="cos_sb", bufs=1)
            nc.scalar.dma_start(cos_sb[:], cosT[:])
            sin_sb = qp.tile([P, S], F32, name="sin_sb", bufs=1)
            nc.scalar.dma_start(sin_sb[:], sinT[:])
            # remaining weights + constants on gpsimd queue (behind hT qc2)
            for gi, (h, pr) in enumerate(groups[2:]):
                chunks = []
                for cs in range(CSUB):
                    w_c = qp.tile([P, CW, P], BF, name=f"w_{pr}{h}_{cs}",
                                  tag="w_sb", bufs=3 * CSUB)
                    nc.gpsimd.dma_start(
                        w_c[:], w_drams[pr][h, :, cs * CW:(cs + 1) * CW, :])
                    chunks.append(w_c)
                w_sbs[(h, pr)] = chunks
            nc.gpsimd.dma_start(bin_sb[:], binT[:])
            nc.gpsimd.dma_start(binN_sb[:], binN[:])
            nc.gpsimd.dma_start(wo_sb[:], wo[:])
            nc.vector.memset(ones_sb[:], 1.0)
            nc.vector.memset(c64_sb[:], float(BS))

            for gi, (h, pr) in enumerate(groups):
                chunks = w_sbs[(h, pr)]
                ps = pq.tile([P, QC, FB], F32, name="ps_qkv", tag="ps_qkv",
                             bufs=2)
                # qc-sequential everywhere: LDWEIGHTS hides at any run length
                # (measured), consumption matches hT arrival at startup, and
                # each qc's eviction overlaps the next qc's matmul chain
                if pr == "v":
                    bsum = qp.tile([P, P], BF, name=f"bsum{h}", bufs=1)
                    nc.vector.memset(bsum[:, NQB:], 0.0)
                for qc in range(QC):
                    for c in range(CH):
                        nc.tensor.matmul(
                            ps[:, qc, :],
                            lhsT=chunks[c // CW][:, c % CW, :],
                            rhs=hT_sbs[(qc, c // CW)][:, c % CW, :],
                            start=(c == 0),
                            stop=(c == CH - 1),
                        )
                    qsl = slice(qc * FB, (qc + 1) * FB)
                    if pr in ("q", "k"):
                        dst = QTr if pr == "q" else KTr
                        tcos = qp.tile([P, FB], F32, name="tcos", tag="tcos")
                        nc.vector.tensor_mul(
                            out=tcos[:], in0=ps[:, qc, :], in1=cos_sb[:, qsl])
                        tsin = qp.tile([P, FB], F32, name="tsin", tag="tsin")
                        nc.vector.tensor_mul(
                            out=tsin[0:64, :], in0=ps[64:128, qc, :],
                            in1=sin_sb[0:64, qsl])
                        nc.vector.tensor_mul(
                            out=tsin[64:128, :], in0=ps[0:64, qc, :],
                            in1=sin_sb[64:128, qsl])
                        nc.vector.tensor_add(
                            out=dst[:, h, qsl], in0=tcos[:], in1=tsin[:])
                    else:
                        vT_c = qp.tile([P, FB], BF, name="vT_c", tag="vT_c",
                                       bufs=4)
                        nc.scalar.copy(out=vT_c[:], in_=ps[:, qc, :])
                        # V natural layout: ONE multi-tile DMA transpose
                        # [128,512] -> 4x[128,128], on the idle sync queue
                        nc.sync.dma_start(
                            V_sbs[h][:, qc * QC:(qc + 1) * QC, :],
                            vT_c[:],
                            transpose=True,
                        )
                        with nc.allow_low_precision(
                                reason="block-sum corr term, 64-wide bf16"):
                            nc.vector.tensor_reduce(
                                out=bsum[:, qc * QB_PER_FB:(qc + 1) * QB_PER_FB],
                                in_=vT_c.rearrange("p (b e) -> p b e", e=BS),
                                axis=mybir.AxisListType.X,
                                op=mybir.AluOpType.add,
                            )
                if pr == "v":
                    # [d, kb-padded] -> [kb-padded, d]; corr matmuls read
                    # partitions 0:NQB only
                    nc.sync.dma_start(corrT_sb[:, h, :], bsum[:],
                                      transpose=True)

            qp.release()
            pq.release()

            # ---------- attention units, fine-grained pipelined emission ----------
            # Per (head, q-chunk) unit: scores -> exp (ACT) -> mask (DVE and
            # gpsimd alternating) -> PV + den chains -> normalize -> o_proj.
            # The PE queue is strictly in-order, so the pending unit's PV/den
            # matmuls are interleaved INTO the next unit's scores stream --
            # otherwise the queue head blocks on the exp eviction and the PE
            # idles ~40% of the attention phase.
            wp = tc.alloc_tile_pool(name="wp", bufs=2)
            pa = tc.alloc_tile_pool(name="pa", bufs=1, space="PSUM")

            NP_ = KT // KTB  # 8 score groups per unit

            units = [(h, qc) for h in range(HPC) for qc in range(QC)]
            pending = None  # state dict of the previous unit

            def emit_scores_group(h, qc, ktp, pts):
                qsl = slice(qc * FB, (qc + 1) * FB)
                qbsl = slice(qc * QB_PER_FB, (qc + 1) * QB_PER_FB)
                ps_s = pa.tile([P, KTB, FB], F32, name="ps_s",
                               tag="ps_s", bufs=2)
                for j in range(KTB):
                    kt = KTB * ktp + j
                    nc.tensor.matmul(
                        ps_s[:, j, :],
                        lhsT=KTr[:, h, kt * P:(kt + 1) * P],
                        rhs=QTr[:, h, qsl],
                        start=True, stop=True,
                    )
                pt = wp.tile([P, KTB, FB], BF, name="probsT",
                             tag="probsT", bufs=2 * NP_)
                nc.scalar.activation(
                    out=pt[:], in_=ps_s[:],
                    func=mybir.ActivationFunctionType.Exp)
                bin_ap = bin_sb[:, h, KTB * ktp:KTB * (ktp + 1), qbsl]
                # early groups masked on the slow-but-idle gpsimd (their
                # results aren't needed until the NEXT unit's window), late
                # groups on the faster DVE (gpsimd is ~2x slower per op)
                eng = nc.gpsimd if ktp < 3 else nc.vector
                eng.tensor_mul(
                    out=pt[:],
                    in0=pt[:],
                    in1=bin_ap[:, :, :, None].to_broadcast(
                        [P, KTB, QB_PER_FB, BS]),
                )
                pts.append(pt)

            def emit_pvden_part(st, ktp):
                # front-loaded: 4 of the PV and den matmuls per slot during
                # slots 0..3, chains closed (corr matmuls) at slot 3
                h, qc, pts = st["h"], st["qc"], st["pts"]
                if ktp == 0:
                    st["ps_o"] = pa.tile([P, FB], F32, name="ps_o",
                                         tag="ps_o", bufs=1)
                    st["ps_d"] = pa.tile([P, FB], F32, name="ps_d",
                                         tag="ps_d", bufs=1)
                for kt in range(4 * ktp, 4 * ktp + 4):
                    nc.tensor.matmul(
                        st["ps_o"][:],
                        lhsT=V_sbs[h][:, kt, :],
                        rhs=pts[kt // KTB][:, kt % KTB, :],
                        start=(kt == 0), stop=False,
                    )
                for kt in range(4 * ktp, 4 * ktp + 4):
                    nc.tensor.matmul(
                        st["ps_d"][:],
                        lhsT=ones_sb[:],
                        rhs=pts[kt // KTB][:, kt % KTB, :],
                        start=(kt == 0), stop=False,
                    )
                if ktp == 3:
                    qbsl = slice(qc * QB_PER_FB, (qc + 1) * QB_PER_FB)
                    binN_ap = binN_sb[:, h, qbsl]
                    bc = binN_ap[:, :, None].to_broadcast(
                        [NQB, QB_PER_FB, BS])
                    nc.tensor.matmul(st["ps_o"][:],
                                     lhsT=corrT_sb[0:NQB, h, :], rhs=bc,
                                     start=False, stop=True)
                    nc.tensor.matmul(st["ps_d"][:], lhsT=c64_sb[:], rhs=bc,
                                     start=False, stop=True)

            def emit_finish(st):
                # normalize on DVE ahead of the next unit's late mask-muls
                h, qc = st["h"], st["qc"]
                rden = wp.tile([P, FB], F32, name="rden", tag="rden")
                nc.vector.reciprocal_approx_fast(out=rden[:], in_=st["ps_d"][:])
                nc.vector.tensor_mul(out=at_sb[:, qc, h, :],
                                     in0=st["ps_o"][:], in1=rden[:])

            def emit_oproj_part(qc, part):
                # 4 strips per slot, spread over slots 4..7
                qsl = slice(qc * FB, (qc + 1) * FB)
                oeng = [nc.sync, nc.scalar, nc.gpsimd, nc.sync]
                for strip in range(4 * part, 4 * part + 4):
                    ps_w = pa.tile([P, FB], F32, name="ps_w", tag="ps_w",
                                   bufs=2)
                    for h in range(HPC):
                        nc.tensor.matmul(
                            ps_w[:],
                            lhsT=wo_sb[:, h, strip, :],
                            rhs=at_sb[:, qc, h, :],
                            start=(h == 0), stop=(h == HPC - 1),
                        )
                    ot = wp.tile([P, FB], BF, name="ot", tag="ot", bufs=4)
                    if strip % 2 == 0:
                        nc.vector.tensor_copy(out=ot[:], in_=ps_w[:])
                    else:
                        nc.scalar.copy(out=ot[:], in_=ps_w[:])
                    oeng[strip % 4].dma_start(out[strip, :, qsl], ot[:])

            oproj_queue = []
            for h, qc in units:
                pts = []
                for ktp in range(NP_):
                    if ktp == 4:
                        if pending is not None:
                            emit_finish(pending)
                            if pending["h"] == HPC - 1:
                                oproj_queue.append(pending["qc"])
                    if ktp >= 4 and oproj_queue:
                        emit_oproj_part(oproj_queue[0], ktp - 4)
                        if ktp == 7:
                            oproj_queue.pop(0)
                    emit_scores_group(h, qc, ktp, pts)
                    if pending is not None and ktp < 4:
                        emit_pvden_part(pending, ktp)
                pending = {"h": h, "qc": qc, "pts": pts}
            # drain the last unit
            for ktp in range(4):
                emit_pvden_part(pending, ktp)
            emit_finish(pending)
            for part in range(4):
                emit_oproj_part(pending["qc"], part)
            wp.release()
            pa.release()

    nc.compile()
    return nc


def _host_prep(hidden_states, q_w, k_w, v_w, o_w, sparsity_pattern):
    hs = np.asarray(hidden_states, dtype=np.float32).reshape(S, HID)
    qw = np.asarray(q_w, dtype=np.float32)
    kw = np.asarray(k_w, dtype=np.float32)
    vw = np.asarray(v_w, dtype=np.float32)
    ow = np.asarray(o_w, dtype=np.float32)
    sp = np.asarray(sparsity_pattern, dtype=np.float32)

    bf = ml_dtypes.bfloat16

    # hidden^T -> [qcb, p, c, s'] (s-chunk-major so chunk DMAs are contiguous)
    hT = np.ascontiguousarray(
        hs.T.reshape(CH, P, QC, FB).transpose(2, 1, 0, 3)).astype(bf)

    # block mask with per-head top-k threshold
    kk = max(1, int(NH * NQB * NQB * RATIO / NH))
    flat = sp.reshape(NH, -1)
    th = np.partition(flat, -kk, axis=1)[:, -kk]
    bm = (sp > th[:, None, None]).astype(np.float32)  # [NH, 32 qb, 32 kb]

    # RoPE tables in [d, s] layout; sin pre-signed for rotate_half
    inv = 1.0 / (THETA ** (np.arange(0, HD, 2, dtype=np.float32) / HD))
    fr = np.arange(S, dtype=np.float32)[:, None] * inv[None, :]  # [S, 64]
    embT = np.ascontiguousarray(np.concatenate([fr, fr], axis=1).T)  # [128,S]
    cosT = np.cos(embT).astype(np.float32)
    sinT = np.sin(embT).astype(np.float32)
    sinT[:64] *= -1.0

    def w_per_head(w, h, scale=1.0):
        # [HID, 128] -> [p, c, d]
        return np.ascontiguousarray(
            (w[:, h * HD:(h + 1) * HD] * scale)
            .reshape(CH, P, HD).transpose(1, 0, 2))

    qscale = 1.0 / np.sqrt(HD)
    in_maps = []
    for r in range(NCORES):
        heads = [HPC * r + i for i in range(HPC)]
        wq_r = np.stack([w_per_head(qw, h, qscale) for h in heads]).astype(bf)
        wk_r = np.stack([w_per_head(kw, h) for h in heads]).astype(bf)
        wv_r = np.stack([w_per_head(vw, h) for h in heads]).astype(bf)
        # o_w rows for this core's heads: [d, h, strip, p] (lhsT tiles,
        # partition dim d first to match the SBUF tile layout)
        wo_r = np.ascontiguousarray(
            ow[r * HPC * HD:(r + 1) * HPC * HD]
            .reshape(HPC, HD, NSTRIP, P).transpose(1, 0, 2, 3)).astype(bf)
        # bm[h] is [q_block, k_block]; kernel layout wants keys on partitions
        mT = np.stack([
            np.repeat(bm[h].T, BS, axis=0).reshape(KT, P, NQB).transpose(1, 0, 2)
            for h in heads
        ], axis=1)  # [P, HPC, KT, NQB]
        # complement mask [kb, h, qb] for the masked-block corrections
        mN = np.stack([1.0 - bm[h].T for h in heads], axis=1)  # [32, HPC, 32]
        in_maps.append({
            "hT": hT,
            "wq": wq_r, "wk": wk_r, "wv": wv_r, "wo": wo_r,
            "cosT": cosT, "sinT": sinT,
            "binT": np.ascontiguousarray(mT).astype(bf),
            "binN": np.ascontiguousarray(mN).astype(bf),
        })
    return in_maps


def _run(inputs, trace=False, **kwargs):
    if "nc" not in _CACHE:
        _CACHE["nc"] = _build()
    nc = _CACHE["nc"]
    in_maps = _host_prep(**inputs)
    res = run_bass_kernel_spmd(
        nc, in_maps, core_ids=list(range(NCORES)), trace=trace, **kwargs)
    # unshard: sum the 8 per-core o_proj partials, then transpose
    acc = np.zeros((HID, S), dtype=np.float32)
    for r in range(NCORES):
        acc += np.asarray(res.results[r]["out"],
                          dtype=np.float32).reshape(HID, S)
    full = np.ascontiguousarray(acc.T).reshape(B, S, HID)
    return full, res


def kernel(**inputs):
    full, _ = _run(inputs, trace=False)
    return full


# revision 31
# speedup vs baseline: 1.1215x; 1.1215x over previous
"""Block-sparse attention (nn_BlockSparseAttention) on 8 TRN2 NeuronCores.

Strategy v2: head-parallel attention (2 heads/core), contraction-sharded
o_proj with NO collectives — each core computes the partial o_proj for
its 2 heads over ALL 2048 output dims and the host sums the 8 partials
(the unshard step). Per core, bf16 on the TensorEngine, f32 PSUM:
  1. QKV projections with the weight stationary across 4 q-chunk
     accumulators (1 LDWEIGHTS per 4 matmuls), RoPE fused into the
     PSUM->SBUF eviction (partition-shifted PSUM reads) on DVE.
  2. Attention per (head, q-chunk) unit in [keys, q] orientation,
     software-pipelined one unit deep so the Scalar-engine exp of unit
     u overlaps the PV/den matmuls of unit u-1. Reference mask
     semantics (masked scores = 0 => exp = 1) via decomposition:
     unmasked exp straight from PSUM (ACT), bf16 mask zeroes unselected
     blocks (DVE), and the "+1 per masked key" terms restored by two
     tiny matmuls (V block-sums x complement mask; 64 x complement
     count) accumulated into the same PSUM groups.
  3. o_proj partials per chunk (16 row strips x 2 head-chunks of
     contraction), evicted bf16 and DMA'd out; no cross-core deps.
Host: input prep (top-k block mask, RoPE tables, layouts), final
sum-of-partials + transpose.
"""
import sys

if "/opt/trn_rl_repo" not in sys.path:
    sys.path.insert(0, "/opt/trn_rl_repo")

import numpy as np
import ml_dtypes

import concourse.bass as bass
import concourse.tile as tile
import concourse.mybir as mybir
from concourse import bacc
from concourse.bass_utils import run_bass_kernel_spmd

# problem constants (hardcoded per harness contract)
B, S, HID = 1, 2048, 2048
NH, HD, BS = 16, 128, 64
RATIO = 0.5
THETA = 10000.0
NCORES = 8
HPC = NH // NCORES          # heads per core = 2
P = 128                     # partitions
CH = HID // P               # contraction chunks = 16
KT = S // P                 # key tiles = 16
FB = 512                    # free-dim block (psum bank)
QC = S // FB                # q chunks = 4
NQB = S // BS               # 32 blocks per side
QB_PER_FB = FB // BS        # 8 q-blocks per 512 chunk
KTB = 2                     # key tiles per exp/mask op
NSTRIP = HID // P           # o_proj output strips = 16
CSUB = 4                    # hT / weight DMA sub-chunks

BF = mybir.dt.bfloat16
F32 = mybir.dt.float32

_CACHE = {}


def _build():
    nc = bacc.Bacc("TRN2", target_bir_lowering=False, debug=False,
                   num_devices=NCORES)

    hT = nc.dram_tensor("hT", [QC, P, CH, FB], BF, kind="ExternalInput").ap()
    wq = nc.dram_tensor("wq", [HPC, P, CH, P], BF, kind="ExternalInput").ap()
    wk = nc.dram_tensor("wk", [HPC, P, CH, P], BF, kind="ExternalInput").ap()
    wv = nc.dram_tensor("wv", [HPC, P, CH, P], BF, kind="ExternalInput").ap()
    wo = nc.dram_tensor("wo", [P, HPC, NSTRIP, P], BF, kind="ExternalInput").ap()
    cosT = nc.dram_tensor("cosT", [P, S], F32, kind="ExternalInput").ap()
    sinT = nc.dram_tensor("sinT", [P, S], F32, kind="ExternalInput").ap()  # pre-signed
    binT = nc.dram_tensor("binT", [P, HPC, KT, NQB], BF, kind="ExternalInput").ap()
    binN = nc.dram_tensor("binN", [NQB, HPC, NQB], BF, kind="ExternalInput").ap()
    out = nc.dram_tensor("out", [NSTRIP, P, S], BF, kind="ExternalOutput").ap()

    with tile.TileContext(nc) as tc:
        with tc.tile_pool(name="cp", bufs=1) as cp:
            QTr = cp.tile([P, HPC, S], BF, name="QTr")
            KTr = cp.tile([P, HPC, S], BF, name="KTr")
            V_sbs = [cp.tile([P, KT, P], BF, name=f"V_h{h}")
                     for h in range(HPC)]
            corrT_sb = cp.tile([P, HPC, P], BF, name="corrT_sb")
            at_sb = cp.tile([P, QC, HPC, FB], BF, name="at_sb")
            wo_sb = cp.tile([P, HPC, NSTRIP, P], BF, name="wo_sb")
            bin_sb = cp.tile([P, HPC, KT, NQB], BF, name="bin_sb")
            binN_sb = cp.tile([NQB, HPC, NQB], BF, name="binN_sb")
            ones_sb = cp.tile([P, P], BF, name="ones_sb")
            c64_sb = cp.tile([NQB, P], BF, name="c64_sb")

            # ---------------- QKV + RoPE (phase-scoped pools) ----------------
            qp = tc.alloc_tile_pool(name="qp", bufs=2)
            pq = tc.alloc_tile_pool(name="pq", bufs=1, space="PSUM")

            # weight chunks: [P, CSUB, P] pieces, 3 projs x 2 heads x 4 chunks
            # group order: V groups LAST -- their evictions are cheap ACT
            # copies (DMA transposes ride the idle sync queue), so attention
            # isn't gated on the slow DVE RoPE evictions of the final groups
            w_drams = {"q": wq, "k": wk, "v": wv}
            groups = [(0, "q"), (0, "k"), (1, "q"), (1, "k"), (0, "v"), (1, "v")]
            w_sbs = {}
            CW = CH // CSUB  # 4 c's per weight chunk
            # first group's weights on the scalar queue, ahead of everything
            for gi, (h, pr) in enumerate(groups[:2]):
                chunks = []
                for cs in range(CSUB):
                    w_c = qp.tile([P, CW, P], BF, name=f"w_{pr}{h}_{cs}",
                                  tag="w_sb", bufs=3 * CSUB)
                    nc.scalar.dma_start(
                        w_c[:], w_drams[pr][h, :, cs * CW:(cs + 1) * CW, :])
                    chunks.append(w_c)
                w_sbs[(h, pr)] = chunks
            # hidden^T: queue per qc, qc-major so the first (qc-sequential)
            # group streams as chunks arrive
            hT_engs = [nc.sync, nc.scalar, nc.gpsimd, nc.sync]
            hT_sbs = {}
            for qc in range(QC):
                for cs in range(CSUB):
                    hT_c = qp.tile([P, CW, FB], BF, name=f"hT_{qc}_{cs}",
                                   bufs=1)
                    hT_engs[qc].dma_start(
                        hT_c[:], hT[qc, :, cs * CW:(cs + 1) * CW, :])
                    hT_sbs[(qc, cs)] = hT_c
            # RoPE tables after the first weights + hT qc1 on scalar
            cos_sb = qp.tile([P, S], F32, name="cos_sb", bufs=1)
            nc.scalar.dma_start(cos_sb[:], cosT[:])
            sin_sb = qp.tile([P, S], F32, name="sin_sb", bufs=1)
            nc.scalar.dma_start(sin_sb[:], sinT[:])
            # remaining weights + constants on gpsimd queue (behind hT qc2)
            for gi, (h, pr) in enumerate(groups[2:]):
                chunks = []
                for cs in range(CSUB):
                    w_c = qp.tile([P, CW, P], BF, name=f"w_{pr}{h}_{cs}",
                                  tag="w_sb", bufs=3 * CSUB)
                    nc.gpsimd.dma_start(
                        w_c[:], w_drams[pr][h, :, cs * CW:(cs + 1) * CW, :])
                    chunks.append(w_c)
                w_sbs[(h, pr)] = chunks
            nc.gpsimd.dma_start(bin_sb[:], binT[:])
            nc.gpsimd.dma_start(binN_sb[:], binN[:])
            nc.gpsimd.dma_start(wo_sb[:], wo[:])
            nc.vector.memset(ones_sb[:], 1.0)
            nc.vector.memset(c64_sb[:], float(BS))

            for gi, (h, pr) in enumerate(groups):
                chunks = w_sbs[(h, pr)]
                ps = pq.tile([P, QC, FB], F32, name="ps_qkv", tag="ps_qkv",
                             bufs=2)
                # first group qc-sequential (consumption matches hT arrival at
                # startup); later groups c-major so 4 consecutive matmuls share
                # one weight load
                if pr == "v":
                    bsum = qp.tile([P, P], BF, name=f"bsum{h}", bufs=1)
                    nc.vector.memset(bsum[:, NQB:], 0.0)
                if gi == 0:
                    for qc in range(QC):
                        for c in range(CH):
                            nc.tensor.matmul(
                                ps[:, qc, :],
                                lhsT=chunks[c // CW][:, c % CW, :],
                                rhs=hT_sbs[(qc, c // CW)][:, c % CW, :],
                                start=(c == 0),
                                stop=(c == CH - 1),
                            )
                else:
                    for c in range(CH):
                        for qc in range(QC):
                            nc.tensor.matmul(
                                ps[:, qc, :],
                                lhsT=chunks[c // CW][:, c % CW, :],
                                rhs=hT_sbs[(qc, c // CW)][:, c % CW, :],
                                start=(c == 0),
                                stop=(c == CH - 1),
                            )
                for qc in range(QC):
                    qsl = slice(qc * FB, (qc + 1) * FB)
                    if pr in ("q", "k"):
                        dst = QTr if pr == "q" else KTr
                        tcos = qp.tile([P, FB], F32, name="tcos", tag="tcos")
                        nc.vector.tensor_mul(
                            out=tcos[:], in0=ps[:, qc, :], in1=cos_sb[:, qsl])
                        tsin = qp.tile([P, FB], F32, name="tsin", tag="tsin")
                        nc.vector.tensor_mul(
                            out=tsin[0:64, :], in0=ps[64:128, qc, :],
                            in1=sin_sb[0:64, qsl])
                        nc.vector.tensor_mul(
                            out=tsin[64:128, :], in0=ps[0:64, qc, :],
                            in1=sin_sb[64:128, qsl])
                        nc.vector.tensor_add(
                            out=dst[:, h, qsl], in0=tcos[:], in1=tsin[:])
                    else:
                        vT_c = qp.tile([P, FB], BF, name="vT_c", tag="vT_c",
                                       bufs=4)
                        nc.scalar.copy(out=vT_c[:], in_=ps[:, qc, :])
                        # V natural layout: ONE multi-tile DMA transpose
                        # [128,512] -> 4x[128,128], on the idle sync queue
                        nc.sync.dma_start(
                            V_sbs[h][:, qc * QC:(qc + 1) * QC, :],
                            vT_c[:],
                            transpose=True,
                        )
                        with nc.allow_low_precision(
                                reason="block-sum corr term, 64-wide bf16"):
                            nc.vector.tensor_reduce(
                                out=bsum[:, qc * QB_PER_FB:(qc + 1) * QB_PER_FB],
                                in_=vT_c.rearrange("p (b e) -> p b e", e=BS),
                                axis=mybir.AxisListType.X,
                                op=mybir.AluOpType.add,
                            )
                if pr == "v":
                    # [d, kb-padded] -> [kb-padded, d]; corr matmuls read
                    # partitions 0:NQB only
                    nc.sync.dma_start(corrT_sb[:, h, :], bsum[:],
                                      transpose=True)

            qp.release()
            pq.release()

            # ---------- attention units, fine-grained pipelined emission ----------
            # Per (head, q-chunk) unit: scores -> exp (ACT) -> mask (DVE and
            # gpsimd alternating) -> PV + den chains -> normalize -> o_proj.
            # The PE queue is strictly in-order, so the pending unit's PV/den
            # matmuls are interleaved INTO the next unit's scores stream --
            # otherwise the queue head blocks on the exp eviction and the PE
            # idles ~40% of the attention phase.
            wp = tc.alloc_tile_pool(name="wp", bufs=2)
            pa = tc.alloc_tile_pool(name="pa", bufs=1, space="PSUM")

            NP_ = KT // KTB  # 8 score groups per unit

            units = [(h, qc) for h in range(HPC) for qc in range(QC)]
            pending = None  # state dict of the previous unit

            def emit_scores_group(h, qc, ktp, pts):
                qsl = slice(qc * FB, (qc + 1) * FB)
                qbsl = slice(qc * QB_PER_FB, (qc + 1) * QB_PER_FB)
                ps_s = pa.tile([P, KTB, FB], F32, name="ps_s",
                               tag="ps_s", bufs=2)
                for j in range(KTB):
                    kt = KTB * ktp + j
                    nc.tensor.matmul(
                        ps_s[:, j, :],
                        lhsT=KTr[:, h, kt * P:(kt + 1) * P],
                        rhs=QTr[:, h, qsl],
                        start=True, stop=True,
                    )
                pt = wp.tile([P, KTB, FB], BF, name="probsT",
                             tag="probsT", bufs=2 * NP_)
                nc.scalar.activation(
                    out=pt[:], in_=ps_s[:],
                    func=mybir.ActivationFunctionType.Exp)
                bin_ap = bin_sb[:, h, KTB * ktp:KTB * (ktp + 1), qbsl]
                # early groups masked on the slow-but-idle gpsimd (their
                # results aren't needed until the NEXT unit's window), late
                # groups on the faster DVE (gpsimd is ~2x slower per op)
                eng = nc.gpsimd if ktp < 3 else nc.vector
                eng.tensor_mul(
                    out=pt[:],
                    in0=pt[:],
                    in1=bin_ap[:, :, :, None].to_broadcast(
                        [P, KTB, QB_PER_FB, BS]),
                )
                pts.append(pt)

            def emit_pvden_part(st, ktp):
                # front-loaded: 4 of the PV and den matmuls per slot during
                # slots 0..3, chains closed (corr matmuls) at slot 3
                h, qc, pts = st["h"], st["qc"], st["pts"]
                if ktp == 0:
                    st["ps_o"] = pa.tile([P, FB], F32, name="ps_o",
                                         tag="ps_o", bufs=1)
                    st["ps_d"] = pa.tile([P, FB], F32, name="ps_d",
                                         tag="ps_d", bufs=1)
                for kt in range(4 * ktp, 4 * ktp + 4):
                    nc.tensor.matmul(
                        st["ps_o"][:],
                        lhsT=V_sbs[h][:, kt, :],
                        rhs=pts[kt // KTB][:, kt % KTB, :],
                        start=(kt == 0), stop=False,
                    )
                for kt in range(4 * ktp, 4 * ktp + 4):
                    nc.tensor.matmul(
                        st["ps_d"][:],
                        lhsT=ones_sb[:],
                        rhs=pts[kt // KTB][:, kt % KTB, :],
                        start=(kt == 0), stop=False,
                    )
                if ktp == 3:
                    qbsl = slice(qc * QB_PER_FB, (qc + 1) * QB_PER_FB)
                    binN_ap = binN_sb[:, h, qbsl]
                    bc = binN_ap[:, :, None].to_broadcast(
                        [NQB, QB_PER_FB, BS])
                    nc.tensor.matmul(st["ps_o"][:],
                                     lhsT=corrT_sb[0:NQB, h, :], rhs=bc,
                                     start=False, stop=True)
                    nc.tensor.matmul(st["ps_d"][:], lhsT=c64_sb[:], rhs=bc,
                                     start=False, stop=True)

            def emit_finish(st):
                # normalize on DVE ahead of the next unit's late mask-muls
                h, qc = st["h"], st["qc"]
                rden = wp.tile([P, FB], F32, name="rden", tag="rden")
                nc.vector.reciprocal_approx_fast(out=rden[:], in_=st["ps_d"][:])
                nc.vector.tensor_mul(out=at_sb[:, qc, h, :],
                                     in0=st["ps_o"][:], in1=rden[:])

            def emit_oproj_part(qc, part):
                # 4 strips per slot, spread over slots 4..7
                qsl = slice(qc * FB, (qc + 1) * FB)
                oeng = [nc.sync, nc.scalar, nc.gpsimd, nc.sync]
                for strip in range(4 * part, 4 * part + 4):
                    ps_w = pa.tile([P, FB], F32, name="ps_w", tag="ps_w",
                                   bufs=2)
                    for h in range(HPC):
                        nc.tensor.matmul(
                            ps_w[:],
                            lhsT=wo_sb[:, h, strip, :],
                            rhs=at_sb[:, qc, h, :],
                            start=(h == 0), stop=(h == HPC - 1),
                        )
                    ot = wp.tile([P, FB], BF, name="ot", tag="ot", bufs=4)
                    if strip % 2 == 0:
                        nc.vector.tensor_copy(out=ot[:], in_=ps_w[:])
                    else:
                        nc.scalar.copy(out=ot[:], in_=ps_w[:])
                    oeng[strip % 4].dma_start(out[strip, :, qsl], ot[:])

            oproj_queue = []
            for h, qc in units:
                pts = []
                for ktp in range(NP_):
                    if ktp == 4:
                        if pending is not None:
                            emit_finish(pending)
                            if pending["h"] == HPC - 1:
                                oproj_queue.append(pending["qc"])
                    if ktp >= 4 and oproj_queue:
                        emit_oproj_part(oproj_queue[0], ktp - 4)
                        if ktp == 7:
                            oproj_queue.pop(0)
                    emit_scores_group(h, qc, ktp, pts)
                    if pending is not None and ktp < 4:
                        emit_pvden_part(pending, ktp)
                pending = {"h": h, "qc": qc, "pts": pts}
            # drain the last unit
            for ktp in range(4):
                emit_pvden_part(pending, ktp)
            emit_finish(pending)
            for part in range(4):
                emit_oproj_part(pending["qc"], part)
            wp.release()
            pa.release()

    nc.compile()
    return nc


def _host_prep(hidden_states, q_w, k_w, v_w, o_w, sparsity_pattern):
    hs = np.asarray(hidden_states, dtype=np.float32).reshape(S, HID)
    qw = np.asarray(q_w, dtype=np.float32)
    kw = np.asarray(k_w, dtype=np.float32)
    vw = np.asarray(v_w, dtype=np.float32)
    ow = np.asarray(o_w, dtype=np.float32)
    sp = np.asarray(sparsity_pattern, dtype=np.float32)

    bf = ml_dtypes.bfloat16

    # hidden^T -> [qcb, p, c, s'] (s-chunk-major so chunk DMAs are contiguous)
    hT = np.ascontiguousarray(
        hs.T.reshape(CH, P, QC, FB).transpose(2, 1, 0, 3)).astype(bf)

    # block mask with per-head top-k threshold
    kk = max(1, int(NH * NQB * NQB * RATIO / NH))
    flat = sp.reshape(NH, -1)
    th = np.partition(flat, -kk, axis=1)[:, -kk]
    bm = (sp > th[:, None, None]).astype(np.float32)  # [NH, 32 qb, 32 kb]

    # RoPE tables in [d, s] layout; sin pre-signed for rotate_half
    inv = 1.0 / (THETA ** (np.arange(0, HD, 2, dtype=np.float32) / HD))
    fr = np.arange(S, dtype=np.float32)[:, None] * inv[None, :]  # [S, 64]
    embT = np.ascontiguousarray(np.concatenate([fr, fr], axis=1).T)  # [128,S]
    cosT = np.cos(embT).astype(np.float32)
    sinT = np.sin(embT).astype(np.float32)
    sinT[:64] *= -1.0

    def w_per_head(w, h, scale=1.0):
        # [HID, 128] -> [p, c, d]
        return np.ascontiguousarray(
            (w[:, h * HD:(h + 1) * HD] * scale)
            .reshape(CH, P, HD).transpose(1, 0, 2))

    qscale = 1.0 / np.sqrt(HD)
    in_maps = []
    for r in range(NCORES):
        heads = [HPC * r + i for i in range(HPC)]
        wq_r = np.stack([w_per_head(qw, h, qscale) for h in heads]).astype(bf)
        wk_r = np.stack([w_per_head(kw, h) for h in heads]).astype(bf)
        wv_r = np.stack([w_per_head(vw, h) for h in heads]).astype(bf)
        # o_w rows for this core's heads: [d, h, strip, p] (lhsT tiles,
        # partition dim d first to match the SBUF tile layout)
        wo_r = np.ascontiguousarray(
            ow[r * HPC * HD:(r + 1) * HPC * HD]
            .reshape(HPC, HD, NSTRIP, P).transpose(1, 0, 2, 3)).astype(bf)
        # bm[h] is [q_block, k_block]; kernel layout wants keys on partitions
        mT = np.stack([
            np.repeat(bm[h].T, BS, axis=0).reshape(KT, P, NQB).transpose(1, 0, 2)
            for h in heads
        ], axis=1)  # [P, HPC, KT, NQB]
        # complement mask [kb, h, qb] for the masked-block corrections
        mN = np.stack([1.0 - bm[h].T for h in heads], axis=1)  # [32, HPC, 32]
        in_maps.append({
            "hT": hT,
            "wq": wq_r, "wk": wk_r, "wv": wv_r, "wo": wo_r,
            "cosT": cosT, "sinT": sinT,
            "binT": np.ascontiguousarray(mT).astype(bf),
            "binN": np.ascontiguousarray(mN).astype(bf),
        })
    return in_maps


def _run(inputs, trace=False, **kwargs):
    if "nc" not in _CACHE:
        _CACHE["nc"] = _build()
    nc = _CACHE["nc"]
    in_maps = _host_prep(**inputs)
    res = run_bass_kernel_spmd(
        nc, in_maps, core_ids=list(range(NCORES)), trace=trace, **kwargs)
    # unshard: sum the 8 per-core o_proj partials, then transpose
    acc = np.zeros((HID, S), dtype=np.float32)
    for r in range(NCORES):
        acc += np.asarray(res.results[r]["out"],
                          dtype=np.float32).reshape(HID, S)
    full = np.ascontiguousarray(acc.T).reshape(B, S, HID)
    return full, res


def kernel(**inputs):
    full, _ = _run(inputs, trace=False)
    return full
